# revision 12
# baseline (speedup 1.0000x reference)
"""GQA multi-head attention (B=2, S=2048, E=2048, 32 q-heads, 8 kv-heads) on 8 TRN2 cores.

Sharding: tensor-parallel over kv-heads (core c owns kv-head c and query heads
4c..4c+3 for both batches). After attention, a per-batch AllToAll re-shards the
context from head-sharding to token-sharding; each core then runs the output
projection for its 256-token slice of each batch against the full Wo. The host
gather is pure concatenation.

All matmul operands are bf16 (fp32 PSUM accumulation); back-to-back bf16
matmuls stream at 1 col/cycle with the weight load hidden, so the kernel is
organized to keep the PE queue saturated: the batch-1 projections and the
batch-0 output projection are emitted as fine-grained fillers inside the
attention instruction streams, filling the PE bubbles that the scalar-engine
exp chain would otherwise create. Causality is structural (above-diagonal
score tiles skipped, diagonal blocks take an additive triangular mask); full
score tiles are computed in pairs so one exp activation covers 1024 columns.
Softmax runs without max-subtraction; row sums ride in the PV matmul via a
ones-column on V; the reciprocal runs batched per head on the scalar engine
(one [1,2048] activation, amortizing the Exp<->Reciprocal table swaps), and the
1/sum broadcast across partitions is a K=1 matmul. Normalization emission is
deferred by one head so no engine queue ever head-of-line blocks on it.
"""

from collections import deque

import numpy as np

B = 2
S = 2048
E = 2048
KV = 8
G = 4
D = 64
H = 32
N_CORES = 8
KC = E // 128   # 16 contraction chunks
NT = S // 512   # 4 q-tiles of 512
NQ = 4          # token quarters for projections (512 each)
TOK = 256       # per-core token slice per batch (out proj)
NTB = S // 128  # 16 token blocks of 128

_CACHE = {}


def _build_nc():
    import concourse.mybir as mybir
    import concourse.tile as tile
    from concourse import bacc
    f32 = mybir.dt.float32
    f32r = mybir.dt.float32r
    bf16 = mybir.dt.bfloat16
    Exp = mybir.ActivationFunctionType.Exp

    nc = bacc.Bacc(target_bir_lowering=False, num_devices=N_CORES)

    xT = nc.dram_tensor("xT", [B, NQ, 128, KC * 512], bf16, kind="ExternalInput")
    wq = nc.dram_tensor("wq", [128, KC * 384], bf16, kind="ExternalInput")
    woT = nc.dram_tensor("woT", [4, 128, KC * 512], bf16, kind="ExternalInput")
    miscb = nc.dram_tensor("miscb", [128, 320], bf16, kind="ExternalInput")
    out = nc.dram_tensor("out", [B * TOK, E], f32, kind="ExternalOutput")

    def act_recip(out_ap, in_ap):
        # Reciprocal on ScalarE (LUT, ~2^-12 rel). Raw emission: the bass
        # wrapper refuses Reciprocal, but one batched [1,2048] activation per
        # head keeps table swaps rare and the DVE free of 3.4us recip ucode.
        eng = nc.scalar
        ins_ = [eng.lower_ap(in_ap)] + [
            mybir.ImmediateValue(dtype=mybir.dt.float32, value=v)
            for v in (0.0, 1.0, 0.0)]
        return eng.add_instruction(mybir.InstActivation(
            name=nc.get_next_instruction_name(),
            func=mybir.ActivationFunctionType.Reciprocal,
            ins=ins_, outs=[eng.lower_ap(out_ap)]))

    with tile.TileContext(nc) as tc:
        with tc.tile_pool(name="const", bufs=1) as const, \
             tc.tile_pool(name="dram", bufs=1, space="DRAM") as dram:

            # ---- constants ----
            miscb_sb = const.tile([128, 320], bf16, name="miscb_sb")
            nc.sync.dma_start(miscb_sb[:], miscb[:])
            ident65 = miscb_sb[0:65, 0:65]
            ones_row = miscb_sb[64:65, 128:192]   # [1, 64] ones (bf16)
            tri01 = miscb_sb[:, 192:320]          # [128,128] 0/1 causal keep-mask

            # ---- collective buffers (per batch) ----
            cc_in = [[dram.tile([N_CORES, 128, TOK], bf16, name=f"cc_in{b}{p}")
                      for p in range(2)] for b in range(B)]
            cc_out = [[dram.tile([N_CORES, 128, TOK], bf16, name=f"cc_out{b}{p}")
                       for p in range(2)] for b in range(B)]

            # ---- long-lived tiles (survive the attention scope) ----
            pre = tc.alloc_tile_pool(name="pre", bufs=1)
            w_sb = pre.tile([128, KC * 384], bf16, name="w_sb")
            # two Wo e-tile slots (et%2), rotated on demand: et order within
            # each out-proj pass touches each et once, so 2 reloads per pass
            woe_sb = [pre.tile([128, KC * 512], bf16, name="woeA"),
                      pre.tile([128, KC * 512], bf16, name="woeB")]
            woe_holds = [None, None]

            def woe_get(et):
                slot = et % 2
                if woe_holds[slot] != et:
                    for g in range(4):
                        nc.sync.dma_start(
                            woe_sb[slot][:, g * 2048:(g + 1) * 2048],
                            woT[et, :, g * 2048:(g + 1) * 2048])
                    woe_holds[slot] = et
                return woe_sb[slot]

            ccr0 = pre.tile([128, KC * TOK], bf16, name="ccr0")
            t1 = pre.tile([128, KC * TOK], bf16, name="t1")
            for g in range(4):
                nc.sync.dma_start(w_sb[:, g * 1536:(g + 1) * 1536],
                                  wq[:, g * 1536:(g + 1) * 1536])
            woe_get(0)

            # ---- filler machinery: generators yielding once per PE op ----
            fillers = deque()

            def pump(k=1):
                done = 0
                while done < k and fillers:
                    try:
                        next(fillers[0])
                        done += 1
                    except StopIteration:
                        fillers.popleft()

            def flush():
                while fillers:
                    pump(64)

            with tc.tile_pool(name="sbA", bufs=1) as sbA, \
                 tc.tile_pool(name="xp", bufs=1) as xpool, \
                 tc.tile_pool(name="sp_ps", bufs=2, space="PSUM") as sp_ps, \
                 tc.tile_pool(name="cx_ps", bufs=2, space="PSUM") as cx_ps, \
                 tc.tile_pool(name="rb_ps", bufs=1, space="PSUM") as rb_ps, \
                 tc.tile_pool(name="fl_ps", bufs=1, space="PSUM") as fl_ps:

                def gen_proj(b):
                    """Projections + v-transposes for batch b. Yields per PE op.
                    Returns tiles via the `proj_out` dict (set on first next())."""
                    q01 = sbA.tile([128, S], bf16, name=f"q01_{b}", tag="q01", bufs=2)
                    q23 = sbA.tile([128, S], bf16, name=f"q23_{b}", tag="q23", bufs=2)
                    kdup = sbA.tile([128, S], bf16, name=f"kd_{b}", tag="kd", bufs=2)
                    vone = sbA.tile([65, S], bf16, name=f"vo_{b}", tag="vo", bufs=2)
                    va = sbA.tile([128, NTB * 65], bf16, name=f"va_{b}", tag="va",
                                  bufs=2)
                    proj_out[b] = (q01, q23, kdup, va)
                    nc.vector.memset(vone[64:65, :], 1.0)
                    dst01 = {0: q01, 1: q23}
                    for tq in range(NQ):
                        xq = xpool.tile([128, KC * 512], bf16, name=f"x{b}{tq}",
                                        tag="x", bufs=2)
                        # batch 0 is latency-critical (nothing overlaps it):
                        # split across 16 issues so the transfers spread over
                        # many DMA queues; batch 1 prefetches under attention
                        nissue = 16 if b == 0 else 4
                        step = KC * 512 // nissue
                        for g in range(nissue):
                            nc.sync.dma_start(
                                xq[:, g * step:(g + 1) * step],
                                xT[b, tq, :, g * step:(g + 1) * step])
                        if b == 1 and tq == 1:
                            # stagger the second Wo e-tile between x streams
                            woe_get(1)
                        for mc in range(3):
                            ps = fl_ps.tile([128, 512], f32, name=f"p{b}{tq}{mc}",
                                            tag="fl")
                            for kc in range(KC):
                                nc.tensor.matmul(
                                    ps[:],
                                    w_sb[:, kc * 384 + mc * 128:
                                         kc * 384 + (mc + 1) * 128],
                                    xq[:, kc * 512:(kc + 1) * 512],
                                    start=(kc == 0), stop=(kc == KC - 1),
                                )
                                yield
                            sl = slice(tq * 512, (tq + 1) * 512)
                            if mc < 2:
                                nc.vector.tensor_copy(dst01[mc][:, sl], ps[:])
                            else:
                                nc.vector.tensor_copy(vone[0:64, sl], ps[0:64, :])
                                nc.vector.tensor_copy(kdup[64:128, sl],
                                                      ps[64:128, :])
                        for t in range(4 * tq, 4 * tq + 4):
                            tp = rb_ps.tile([128, 65], bf16, name=f"vt{b}{t}",
                                            tag="rb")
                            nc.tensor.transpose(
                                tp[:], vone[0:65, t * 128:(t + 1) * 128], ident65)
                            yield
                            nc.vector.tensor_copy(va[:, t * 65:(t + 1) * 65], tp[:])
                    # k onto partitions 0:64 (cross-partition: DMA)
                    nc.gpsimd.dma_start(kdup[0:64, :], kdup[64:128, :])

                def stage_a2a(b, p):
                    for j in range(N_CORES):
                        for lane in range(2):
                            h = 2 * p + lane
                            nc.gpsimd.dma_start(
                                cc_in[b][p][j, lane * 64:(lane + 1) * 64, :],
                                ctx_heads[(b, h)][:, j * TOK:(j + 1) * TOK],
                            )
                    nc.gpsimd.collective_compute(
                        "AllToAll",
                        mybir.AluOpType.bypass,
                        replica_groups=[list(range(N_CORES))],
                        ins=[cc_in[b][p][:]],
                        outs=[cc_out[b][p][:]],
                    )
                    dstt = ccr0 if b == 0 else t1
                    for j in range(N_CORES):
                        fc = 2 * j + p
                        nc.gpsimd.dma_start(
                            dstt[:, fc * TOK:(fc + 1) * TOK],
                            cc_out[b][p][j, :, :])

                ctx_heads = {}
                proj_out = {}

                def gen_outproj(bb, ctx, parity, psum_pool, sb_pool, et_order,
                                psum_bufs=1):
                    os_tiles = [sb_pool.tile([128, E], f32, name=f"os{bb}{mt}",
                                             tag="os", bufs=2)
                                for mt in range(TOK // 128)]
                    fcs = ([fc for fc in range(KC) if fc % 2 == parity] +
                           [fc for fc in range(KC) if fc % 2 != parity])
                    for ei, et in enumerate(et_order):
                        wt = woe_get(et)
                        for mt in range(TOK // 128):
                            ps = psum_pool.tile([128, 512], f32,
                                                name=f"o{bb}{et}{mt}", tag="fl",
                                                bufs=psum_bufs)
                            for i, fc in enumerate(fcs):
                                nc.tensor.matmul(
                                    ps[:],
                                    ctx[:, fc * TOK + mt * 128:
                                        fc * TOK + (mt + 1) * 128],
                                    wt[:, fc * 512:(fc + 1) * 512],
                                    start=(i == 0), stop=(i == KC - 1),
                                )
                                yield
                            nc.vector.tensor_copy(
                                os_tiles[mt][:, et * 512:(et + 1) * 512], ps[:])
                            if ei == 3:
                                nc.sync.dma_start(
                                    out[bb * TOK + mt * 128:
                                        bb * TOK + (mt + 1) * 128, :],
                                    os_tiles[mt][:])

                def attention(b, order, stage_after, activate):
                    q01, q23, kdup, va = proj_out[b]
                    qsrc = {0: q01, 1: q01, 2: q23, 3: q23}
                    pending = [None]
                    for hi, h in enumerate(order):
                        if hi in activate:
                            fillers.append(activate[hi]())
                        base = 64 * (h % 2)
                        ctxf = sbA.tile([65, S], f32, name=f"cf{b}{h}", tag="ctxf",
                                        bufs=2)
                        rcb = sbA.tile([65, S], bf16, name=f"rcb{b}{h}", tag="rcb",
                                       bufs=2)
                        ch = sbA.tile([64, S], bf16, name=f"ctx{b}{h}", tag="ch",
                                      bufs=5)
                        ctx_heads[(b, h)] = ch
                        for qt in range(NT):
                            cps = cx_ps.tile([65, 512], f32, name=f"c{b}{h}{qt}",
                                             tag="cx")
                            nkb = 4 * qt + 4
                            fulls = list(range(4 * qt))
                            groups = [fulls[i:i + 2]
                                      for i in range(0, len(fulls), 2)]
                            groups += [[kb] for kb in range(4 * qt, nkb)]
                            for grp in groups:
                                sp = sp_ps.tile([128, 1024], f32,
                                                name=f"s{b}{h}{qt}{grp[0]}",
                                                tag="sp")
                                offs = []
                                for j, kb in enumerate(grp):
                                    r = kb - 4 * qt
                                    off = 128 * r if r >= 0 else 0
                                    w = 512 - off
                                    nc.tensor.matmul(
                                        sp[:, j * 512: j * 512 + w],
                                        kdup[base:base + 64,
                                             kb * 128:(kb + 1) * 128],
                                        qsrc[h][base:base + 64,
                                                qt * 512 + off:(qt + 1) * 512],
                                        start=True, stop=True,
                                    )
                                    pump(1)
                                    offs.append((kb, off, w, r))
                                span = 512 * (len(grp) - 1) + offs[-1][2]
                                ex = sbA.tile([128, 1024], bf16,
                                              name=f"e{b}{h}{qt}{grp[0]}",
                                              tag="ex", bufs=3)
                                nc.scalar.activation(ex[:, 0:span], sp[:, 0:span],
                                                     Exp, scale=0.125)
                                for j, (kb, off, w, r) in enumerate(offs):
                                    if r >= 0:
                                        # zero the above-diagonal exp block; on
                                        # GpSimd (SBUF-only) so neither the DVE
                                        # nor scalar queue sits in this chain
                                        nc.gpsimd.tensor_mul(
                                            ex[:, j * 512: j * 512 + 128],
                                            ex[:, j * 512: j * 512 + 128], tri01)
                                for j, (kb, off, w, r) in enumerate(offs):
                                    nc.tensor.matmul(
                                        cps[:, off:off + w],
                                        va[:, kb * 65:(kb + 1) * 65],
                                        ex[:, j * 512: j * 512 + w],
                                        start=(kb == 0), stop=(kb == nkb - 1),
                                    )
                                    pump(1)
                            nc.vector.tensor_copy(
                                ctxf[:, qt * 512:(qt + 1) * 512], cps[:])
                            if qt == 1 and pending[0] is not None:
                                pending[0]()
                                pending[0] = None
                        act_recip(rcb[64:65, :], ctxf[64:65, :])

                        def make_norm(ctxf=ctxf, rcb=rcb, ch=ch, h=h):
                            def norm():
                                for qt in range(NT):
                                    sl = slice(qt * 512, (qt + 1) * 512)
                                    rbp = rb_ps.tile([64, 512], f32,
                                                     name=f"rb{b}{h}{qt}",
                                                     tag="rb")
                                    nc.tensor.matmul(rbp[:], ones_row,
                                                     rcb[64:65, sl],
                                                     start=True, stop=True)
                                    nc.vector.tensor_mul(ch[:, sl],
                                                         ctxf[0:64, sl], rbp[:])
                                if h in stage_after:
                                    stage_after[h]()
                            return norm

                        pending[0] = make_norm()
                        if hi == len(order) - 1:
                            pending[0]()
                            pending[0] = None

                # ---- batch 0: projections (unfilled), then attention with
                # batch-1 projections as PE fillers ----
                for _ in gen_proj(0):
                    pass
                fillers.append(gen_proj(1))
                attention(0, [0, 1, 2, 3],
                          stage_after={1: lambda: stage_a2a(0, 0),
                                       3: lambda: stage_a2a(0, 1)},
                          activate={})
                flush()
                # ---- batch 1 attention (kv-pair heads first so the final
                # AllToAll is the p0 one), batch-0 out-proj as fillers from the
                # second head on (by then the b0/p1 AllToAll has landed) ----
                attention(1, [2, 3, 0, 1],
                          stage_after={3: lambda: stage_a2a(1, 1),
                                       1: lambda: stage_a2a(1, 0)},
                          activate={1: lambda: gen_outproj(
                              0, ccr0, 0, fl_ps, sbA, [0, 1, 2, 3])})
                flush()

            # ---- batch-1 out projection (tail): odd fc chunks first (their
            # AllToAll landed mid-attention); evens wait on the final one ----
            with tc.tile_pool(name="sbB", bufs=1) as sbB, \
                 tc.tile_pool(name="op_ps", bufs=1, space="PSUM") as op_ps:
                for _ in gen_outproj(1, t1, 1, op_ps, sbB, [2, 3, 0, 1],
                                     psum_bufs=6):
                    pass
            pre.release()

    nc.compile()
    return nc


def _prep_inputs(x, Wq, Wk, Wv, Wo):
    """Host-side sharding/layout. Returns per-core in_maps."""
    import ml_dtypes
    bf = ml_dtypes.bfloat16
    # xT[b, tq, p, kc*512+t] = x[b, tq*512+t, kc*128+p]
    xT = np.ascontiguousarray(
        x.reshape(B, NQ, 512, KC, 128).transpose(0, 1, 4, 3, 2)
    ).reshape(B, NQ, 128, KC * 512).astype(bf)
    # woT[et, p, kc*512+c] = Wo.T[kc*128+p, et*512+c]
    woT = np.ascontiguousarray(
        Wo.T.reshape(KC, 128, 4, 512).transpose(2, 1, 0, 3)
    ).reshape(4, 128, KC * 512).astype(bf)
    # multiplicative causal keep-mask for transposed scores: 1 where kv<=q
    tri01 = np.triu(np.ones((128, 128), dtype=np.float32))
    ident = np.eye(128, dtype=np.float32)
    ones = np.ones((128, 64), dtype=np.float32)
    miscb = np.ascontiguousarray(
        np.concatenate([ident, ones, tri01], axis=1)).astype(bf)
    in_maps = []
    for c in range(N_CORES):
        wc = np.concatenate([
            Wq[256 * c:256 * (c + 1)],          # q heads 4c..4c+3 -> rows 0..255
            Wv[64 * c:64 * (c + 1)],            # v                -> rows 256..319
            Wk[64 * c:64 * (c + 1)],            # k                -> rows 320..383
        ], axis=0)                              # [384, E]
        # wq[p, kc*384+j] = wc.T[kc*128+p, j]
        wq2 = np.ascontiguousarray(
            wc.T.reshape(KC, 128, 384).transpose(1, 0, 2)
        ).reshape(128, KC * 384).astype(bf)
        in_maps.append({"xT": xT, "wq": wq2, "woT": woT, "miscb": miscb})
    return in_maps


def _ensure_ntff_hook():
    """Install antenv.axon_hooks shim so trace=True can capture NTFF profiles."""
    import sys
    import types
    try:
        from antenv.axon_hooks import get_axon_ntff_profile_hook  # noqa: F401
        return
    except ImportError:
        pass
    mod = types.ModuleType("antenv.axon_hooks")
    _h = [None]
    mod.set_axon_ntff_profile_hook = lambda h: _h.__setitem__(0, h)
    mod.get_axon_ntff_profile_hook = lambda: _h[0]
    sys.modules["antenv.axon_hooks"] = mod
    try:
        from trn_agent_boot.trn_boot import _ntff_profile_via_ctypes
        hook = _ntff_profile_via_ctypes("/opt/axon/libaxon_pjrt.so")
        if hook is not None:
            mod.set_axon_ntff_profile_hook(hook)
    except Exception:
        pass


def kernel(x, mask, Wq, Wk, Wv, Wo, trace=False):
    from concourse.bass_utils import run_bass_kernel_spmd
    if trace:
        _ensure_ntff_hook()

    x = np.asarray(x, dtype=np.float32)
    Wq = np.asarray(Wq, dtype=np.float32)
    Wk = np.asarray(Wk, dtype=np.float32)
    Wv = np.asarray(Wv, dtype=np.float32)
    Wo = np.asarray(Wo, dtype=np.float32)

    if "nc" not in _CACHE:
        _CACHE["nc"] = _build_nc()
    nc = _CACHE["nc"]

    in_maps = _prep_inputs(x, Wq, Wk, Wv, Wo)
    res = run_bass_kernel_spmd(nc, in_maps, core_ids=list(range(N_CORES)),
                               trace=trace)
    _CACHE["last_result"] = res

    full = np.empty((B, S, E), dtype=np.float32)
    for c in range(N_CORES):
        o = res.results[c]["out"]  # [B*TOK, E]
        for b in range(B):
            full[b, TOK * c:TOK * (c + 1), :] = o[b * TOK:(b + 1) * TOK]
    return full
